# revision 5
# baseline (speedup 1.0000x reference)
"""Kent-distribution pairwise KLD loss kernel for Trainium2 (8 NeuronCores).

The [N, M] pairwise KLD matrix factors exactly as a rank-11 product
U @ V^T; N (pred rows) is sharded across the 8 cores.  The pairwise
part runs as ONE bf16 matmul per 512-column chunk with contraction
K=15:

  V15 = [Vh(11); Vl(f1..4)]     U15 = [Uh(11); Uh(f1..4)]
  sum_k U15[k] V15[k] = Uh . Vh + Uh_{f1..4} . Vl_{f1..4}

i.e. split-float bf16 hi+lo on the V side for the four large features
(c_b, -k*gamma_b1); U and the six beta features are bf16-only (total
quantization ~5e-3 rel vs the 2e-2 gate).  The output is written to
HBM as bf16 (halves the dominant output-DMA stream) and widened to
fp32 on the host.

Algebraic reductions vs the reference (see git history for the full
derivation): l1 = (k^2-k-s)/D, l2-free UF[5:8] via unit gammas,
LN_2PI cancellation, G3 sign fold, cos(x) = sin(pi/2 - |x|).

Scheduling (engine queues are strictly in-order):
 - targ input DMA on the Sync HWDGE ring, pred on the GpSimd SWDGE
   ring: parallel issue, and the ACT queue holds no DMA before its
   activations so the trig table loads ONCE during the DMA wait.
 - The transpose->SBUF copies un-interleave VT into m-major order so
   the main matmuls stream a CONTIGUOUS moving operand (measured 2x
   on the per-block matmul cadence vs the strided AP).
 - Elementwise chain packed via strided/negative-stride APs over one
   workspace tile W (slot axis x 18 group columns), split DVE/Pool/ACT
   as in the measured-tight baseline schedule; quadratics and features
   run per 8-group half so the first transposes start early; squares
   ride the post-Ln ACT gap.
 - PSUM->SBUF output copies split DVE(336)/ACT(176); ONE bf16 DMA per
   512-col c-chunk covering both row tiles (descriptor-gen off the
   block cadence); the final chunk ships as two per-row-tile DMAs so
   the last transfer is small and issues immediately.
"""

import sys

import numpy as np

sys.path.insert(0, "/opt/trn_rl_repo")

import concourse.bass as bass  # noqa: E402,F401
import concourse.mybir as mybir  # noqa: E402
import concourse.tile as tile  # noqa: E402
from concourse import bacc  # noqa: E402
from concourse.masks import make_identity  # noqa: E402

F32 = mybir.dt.float32
BF16 = mybir.dt.bfloat16
AF = mybir.ActivationFunctionType
ALU = mybir.AluOpType

N = 2048
M = 2048
NCORES = 8
NS = N // NCORES
K = 11
KL = 4  # V features keeping a bf16 lo part (f1..f4: c_b, -k*g_b1)
KH = K + KL  # 15: contraction size [Vh(11); Vl(4)]
GP = NS // 128  # 2
GT = M // 128  # 16
G = GP + GT  # 18

PI = float(np.pi)
EPS = 1e-6


def _body(tc, pred, targ, out):
    nc = tc.nc
    with (
        tc.tile_pool(name="main", bufs=1) as pool,
        tc.tile_pool(name="vt_psum", bufs=3, space="PSUM") as vpp,
        tc.tile_pool(name="ut_psum", bufs=1, space="PSUM") as upp,
        tc.tile_pool(name="out_psum", bufs=4, space="PSUM") as opp,
    ):
        def t(shape, tag, dtype=F32):
            return pool.tile([128, *shape], dtype, name=tag, tag=tag)

        dve = nc.vector
        act = nc.scalar
        gps = nc.gpsimd

        # ---- input DMAs first: targ on Sync (HWDGE), pred on GpSimd
        # (SWDGE) so they issue in parallel and ACT stays DMA-free ----
        params = t([G * 5], "params")
        nc.sync.dma_start(
            out=params[:, GP * 5 : G * 5],
            in_=targ.rearrange("(p j) c -> p (j c)", p=128),
        )
        gps.dma_start(
            out=params[:, 0 : GP * 5],
            in_=pred.rearrange("(p j) c -> p (j c)", p=128),
        )

        P5 = params.rearrange("p (g c) -> p c g", c=5)
        kap = P5[:, 3, :]
        bet = P5[:, 4, :]
        kap_p, bet_p = kap[:, 0:GP], bet[:, 0:GP]
        kap_t, bet_t = kap[:, GP:G], bet[:, GP:G]

        # ---- constants ----
        half_pi = pool.tile([128, 1], F32, name="half_pi", tag="half_pi")
        gps.memset(half_pi, PI / 2)
        eps_c = pool.tile([128, 1], F32, name="eps_c", tag="eps_c")
        gps.memset(eps_c, EPS)
        ident = pool.tile([128, 128], BF16, name="ident", tag="ident")
        make_identity(nc, ident)

        # dummy sin: trig ACT table loads during the input DMA
        dmy = pool.tile([128, 1], F32, name="dmy", tag="dmy")
        act.activation(dmy[:], half_pi[:], AF.Sin)

        # ---- workspace W: slot axis x 18 group columns.
        # 0 ce, 1 ca, 2 cp | 3 se, 4 sa, 5 sp | 6 g1x, 7 g1y, 8 g1z |
        # 9 m2, 10 m4 | 11 spce, 12 spse, 13 cpce, 14 cpse |
        # 15 m2ce, 16 m2se, 17 m4ce, 18 m4se |
        # 19 g2x, 20 g2y, 21 g2z | 22 G3x, 23 G3y, 24 G3z |
        # 25:31 squares [g2,G3] | 31:37 offdiags | 37:40 dVdiag,
        # 40:43 dVoff | 43 km, 44 kp, 45 LNIN, 46 lnprod |
        # 47:53 p1 (pred cols) | 58:61 absv | 61 b2
        S = 62
        W = t([S, G], "W")

        U = t([20, GP], "U")

        def u(i):
            return U[:, i, :]

        # ---- Pool: kappa/beta shared (needs only the DMA) ----
        gps.tensor_add(W[:, 61, :], bet, bet)  # b2 = 2*beta
        gps.tensor_sub(W[:, 43, :], kap, W[:, 61, :])  # km
        gps.tensor_add(W[:, 44, :], kap, W[:, 61, :])  # kp
        gps.tensor_mul(W[:, 45, :], W[:, 43, :], W[:, 44, :])  # LNIN

        # ---- DVE: abs + U-chain head (only needs the DMA) ----
        angles = P5[:, 0:3, :]
        absv = W[:, 58:61, :]
        dve.scalar_tensor_tensor(absv, angles, -1.0, angles, ALU.mult, ALU.max)
        dve.tensor_mul(u(0), kap_p, kap_p)  # x2
        dve.scalar_tensor_tensor(u(1), bet_p, 4.0, bet_p, ALU.mult, ALU.mult)  # s
        dve.tensor_sub(u(2), u(0), u(1))  # D
        dve.reciprocal(u(3), u(2))  # rec

        # ---- ACT: trig + ln (one ln-table load in between) ----
        act.activation(W[:, 3:6, :], angles, AF.Sin)  # se, sa, sp
        act.activation(W[:, 0:3, :], absv, AF.Sin, bias=half_pi, scale=-1.0)
        act.activation(W[:, 46, :], W[:, 45, :], AF.Ln, bias=eps_c)  # lnprod

        def rep_outer(ap, n):
            return ap.unsqueeze(2).broadcast_to([128, ap.shape[1], n, ap.shape[2]])

        def rep_inner(ap, n):
            return ap.unsqueeze(1).broadcast_to([128, n, ap.shape[1], ap.shape[2]])

        # ---- DVE: gamma chain ----
        cpsp = W[:, 2:6:3, :]
        cese = W[:, 0:4:3, :]
        sa_b = W[:, 4:5, :].broadcast_to([128, 2, G])
        dve.tensor_mul(W[:, 9:11, :], cpsp, W[:, 1:2, :].broadcast_to([128, 2, G]))
        dve.tensor_copy(W[:, 6:7, :], W[:, 1:2, :])
        dve.tensor_mul(W[:, 7:9, :], sa_b, cese)
        aux1_out = W[:, 11:15, :].rearrange("p (a b) g -> p a b g", a=2)
        dve.tensor_mul(aux1_out, rep_outer(W[:, 5:1:-3, :], 2), rep_inner(cese, 2))
        aux2_out = W[:, 15:19, :].rearrange("p (a b) g -> p a b g", a=2)
        dve.tensor_mul(aux2_out, rep_outer(W[:, 9:11, :], 2), rep_inner(cese, 2))
        dve.scalar_tensor_tensor(W[:, 19:23:3, :], cpsp, -1.0, sa_b, ALU.mult, ALU.mult)
        dve.tensor_sub(W[:, 20:25:4, :], W[:, 15:19:3, :], W[:, 12:14, :])
        dve.tensor_add(W[:, 21:24:2, :], W[:, 16:18, :], W[:, 11:15:3, :])

        # ---- p1 products (Pool) + V pair products ----
        g1p = W[:, 6:9, 0:GP]
        gps.tensor_mul(W[:, 47:50, 0:GP], g1p, g1p)
        gps.tensor_mul(
            W[:, 50:52, 0:GP],
            W[:, 6:7, 0:GP].broadcast_to([128, 2, GP]),
            W[:, 7:9, 0:GP],
        )
        gps.tensor_mul(W[:, 52, 0:GP], W[:, 7, 0:GP], W[:, 8, 0:GP])
        # quadratic products split into the two 8-group target halves so
        # the first transposes start as soon as half 0's features land;
        # pred columns are dead here (U uses the p1 products) and dropped.
        # squares ride ACT ('square' is in every table set - no reload).
        for gl, gr in ((GP, GP + 8), (GP + 8, G)):
            act.square(W[:, 25:31, gl:gr], W[:, 19:25, gl:gr])
        for gl, gr in ((GP, GP + 8), (GP + 8, G)):
            dve.tensor_mul(  # g2xy, g2xz
                W[:, 31:33, gl:gr],
                W[:, 19:20, gl:gr].broadcast_to([128, 2, 8]),
                W[:, 20:22, gl:gr],
            )
        for gl, gr in ((GP, GP + 8), (GP + 8, G)):
            gps.tensor_mul(  # G3xy, G3xz
                W[:, 34:36, gl:gr],
                W[:, 22:23, gl:gr].broadcast_to([128, 2, 8]),
                W[:, 23:25, gl:gr],
            )
            gps.tensor_mul(  # yz
                W[:, 33:39:3, gl:gr], W[:, 20:26:3, gl:gr], W[:, 21:27:3, gl:gr]
            )

        # ---- DVE: U-chain middle + dV subs ----
        dve.tensor_sub(u(5), u(2), kap_p)  # n2 = D - k
        dve.tensor_mul(u(6), u(5), u(3))  # l1
        dve.scalar_tensor_tensor(u(7), kap_p, -1.0, u(0), ALU.add, ALU.mult)  # t2
        dve.scalar_tensor_tensor(u(8), kap_p, 0.5, u(1), ALU.add, ALU.mult)  # ks+s/2
        dve.tensor_sub(u(9), u(7), u(8))  # Qh
        dve.tensor_mul(u(10), u(3), u(3))  # rec^2
        dve.tensor_mul(u(11), u(9), u(10))  # l2
        dve.tensor_sub(u(12), u(6), u(11))  # dE
        dve.tensor_mul(u(13), kap_p, u(6))  # kadot

        # ---- V features: f1-f4 (c_b, -k*g_b1) get bf16 hi/lo; the six
        # beta features go straight to bf16.  V15 = [Vh(11); Vl(4)]. ----
        VH = t([KH, GT], "VH", BF16)
        UH = t([KH, GP], "UH", BF16)
        gps.memset(VH[:, 0, :], 1.0)  # V feature 0 == 1 (exact in bf16)
        for h in range(2):
            gl, gr = GP + 8 * h, GP + 8 * h + 8
            vl, vr = 8 * h, 8 * h + 8
            ktb = kap_t[:, vl:vr].unsqueeze(1).broadcast_to([128, 3, 8])
            btb = bet_t[:, vl:vr].unsqueeze(1).broadcast_to([128, 3, 8])
            # dV for half 0 on DVE (queue-adjacent to its features);
            # half 1's on Pool so half 0's feature tail is not blocked
            eng = dve if h == 0 else gps
            eng.tensor_sub(W[:, 37:40, gl:gr], W[:, 28:31, gl:gr], W[:, 25:28, gl:gr])
            eng.tensor_sub(W[:, 40:43, gl:gr], W[:, 34:37, gl:gr], W[:, 31:34, gl:gr])
            dve.scalar_tensor_tensor(
                W[:, 48, gl:gr], W[:, 46, gl:gr], -0.5, kap_t[:, vl:vr],
                ALU.mult, ALU.add,
            )
            dve.scalar_tensor_tensor(
                W[:, 49:52, gl:gr], W[:, 6:9, gl:gr], -1.0, ktb,
                ALU.mult, ALU.mult,
            )
            gps.tensor_mul(VH[:, 5:8, vl:vr], W[:, 37:40, gl:gr], btb)
            dve.scalar_tensor_tensor(
                VH[:, 8:11, vl:vr], W[:, 40:43, gl:gr], 2.0, btb,
                ALU.mult, ALU.mult,
            )
            dve.tensor_copy(VH[:, 1:5, vl:vr], W[:, 48:52, gl:gr])  # hi f1-4
            dve.tensor_sub(  # lo f1-4
                VH[:, K : K + KL, vl:vr],
                W[:, 48:52, gl:gr],
                VH[:, 1:5, vl:vr],
            )
        # A = 0.5*lnprod - k + k*l1
        dve.scalar_tensor_tensor(
            u(14), W[:, 46, 0:GP], 0.5, kap_p, ALU.mult, ALU.subtract
        )

        # ---- Pool: U features, written directly in bf16 ----
        gps.memset(UH[:, 1:12:10, :], 1.0)  # U f1 hi == U (f1 dup) == 1
        l1b = U[:, 6:7, :].broadcast_to([128, 3, GP])
        deb = U[:, 12:13, :].broadcast_to([128, 3, GP])
        de2 = U[:, 12:13, :].broadcast_to([128, 2, GP])
        gps.tensor_mul(UH[:, 2:5, :], g1p, l1b)
        gps.tensor_mul(UH[:, 5:8, :], W[:, 47:50, 0:GP], deb)
        gps.tensor_mul(UH[:, 8:10, :], W[:, 50:52, 0:GP], de2)
        gps.tensor_mul(UH[:, 10, :], W[:, 52, 0:GP], U[:, 12, :])
        gps.tensor_add(UH[:, 0, :], u(14), u(13))
        gps.tensor_copy(UH[:, 12:15, :], UH[:, 2:5, :])  # dup f2-4 = Ex_a

        # ---- PE: V transposes q0..q3 then U transposes.  The psum->SBUF
        # copies un-interleave VT into m-major order (dst runs of 4x2B at
        # 32B stride) so the main matmuls stream a CONTIGUOUS moving
        # operand; VH-gated dummies keep HAM warm through this phase ----
        VT = pool.tile([KH, M], BF16, name="VT", tag="VT")
        VTw = VT.rearrange("k (p j) -> k p j", j=16)  # m = 16p + j
        utp = upp.tile([KH, 256], BF16, name="utp", tag="utp")
        UT = pool.tile([KH, NS], BF16, name="UT", tag="UT")
        for q in range(4):
            vtp = vpp.tile([KH, 1024], BF16, name="vtp", tag="vtp")

            for jj in range(4):
                j = q * 4 + jj
                nc.tensor.transpose(
                    vtp[:, jj * 128 : (jj + 1) * 128], VH[:, :, j], ident[:]
                )
            vtps = vtp[:, 0:512].rearrange("k (j p) -> k p j", p=128)
            # ACT takes the bigger share: DVE is still finishing the
            # half-1 feature tail when the early transposes land
            dve.tensor_copy(VTw[:, 0:48, 4 * q : 4 * q + 4], vtps[:, 0:48, :])
            act.copy(VTw[:, 48:128, 4 * q : 4 * q + 4], vtps[:, 48:128, :])
        # U transposes last (the Pool U-tail lands later than the V halves);
        # UT copy on DVE, which drains its small q-shares well before ACT
        for j in range(GP):
            nc.tensor.transpose(utp[:, j * 128 : (j + 1) * 128], UH[:, :, j], ident[:])
        dve.tensor_copy(
            UT.rearrange("k (p j) -> k j p", j=GP),
            utp[:].rearrange("k (j p) -> k j p", p=128),
        )

        # ---- main matmuls (bf16, K=15): contiguous moving operand.  One
        # DMA per 512-col c-chunk covering both row tiles keeps the Sync
        # descriptor-gen queue (~620ns each) off the block cadence; the
        # final chunk drains in 128-col quarters on both rings ----
        outv = out.rearrange("(t p) m -> p t m", p=128)
        for c in range(4):
            osb = pool.tile([128, GP, 512], BF16, name="osb", tag="osb", bufs=4)
            for ti in range(GP):
                ops = opp.tile([128, 512], F32, name="ops", tag="ops")
                nc.tensor.matmul(
                    ops[:],
                    UT[:, 128 * ti : 128 * (ti + 1)],
                    VT[:, 512 * c : 512 * (c + 1)],
                    start=True,
                    stop=True,
                )
                dve.tensor_copy(osb[:, ti, 0:336], ops[:, 0:336])
                act.copy(osb[:, ti, 336:512], ops[:, 336:512])
                if c == 3:
                    # final chunk: per-row-tile DMAs so the last transfer
                    # is small and issues as soon as its copies land
                    nc.sync.dma_start(
                        out=outv[:, ti, 512 * c : 512 * (c + 1)],
                        in_=osb[:, ti, :],
                    )
            if c < 3:
                nc.sync.dma_start(
                    out=outv[:, :, 512 * c : 512 * (c + 1)], in_=osb[:]
                )


def build():
    nc = bacc.Bacc()
    pred = nc.dram_tensor("pred", [NS, 5], F32, kind="ExternalInput")
    targ = nc.dram_tensor("targ", [M, 5], F32, kind="ExternalInput")
    out = nc.dram_tensor("out", [NS, M], BF16, kind="ExternalOutput")
    with tile.TileContext(nc) as tc:
        _body(tc, pred[:], targ[:], out[:])
    nc.finalize()
    return nc


_NC_CACHE = None


def _get_nc():
    global _NC_CACHE
    if _NC_CACHE is None:
        _NC_CACHE = build()
    return _NC_CACHE


def kernel(kent_pred, kent_target, trace=False, tmpdir=None):
    from concourse.bass_utils import run_bass_kernel_spmd

    nc = _get_nc()
    kent_pred = np.ascontiguousarray(np.asarray(kent_pred, dtype=np.float32))
    kent_target = np.ascontiguousarray(np.asarray(kent_target, dtype=np.float32))
    in_maps = [
        {"pred": kent_pred[i * NS : (i + 1) * NS], "targ": kent_target}
        for i in range(NCORES)
    ]
    res = run_bass_kernel_spmd(
        nc, in_maps, core_ids=list(range(NCORES)), trace=trace, tmpdir=tmpdir
    )
    out = np.concatenate(
        [np.asarray(r["out"]).astype(np.float32) for r in res.results], axis=0
    )
    if trace:
        kernel.last_results = res
    return out
